# revision 28
# baseline (speedup 1.0000x reference)
"""DeepONet forward + JVPs on 8 Trainium2 NeuronCores (Bass/Tile).

Math (see reference):
  branch MLP (4x tanh layers, in_dim 1)  -> basis_br, dbr_mu   [b, 128]
  trunk  MLP (4x tanh layers, in_dim 2)  -> basis_tr, dtr_x, dtr_y  [e, 128]
  A  = basis_br * w_lin.T ; Ap = dbr_mu * w_lin.T
  U     = A  @ basis_tr.T      dU_x = A @ dtr_x.T
  dU_y  = A  @ dtr_y.T         dU_mu = Ap @ basis_tr.T
  outputs each [b, e, 1] f32

Sharding: 2x4 grid over (b, e): each core computes a [2048, 1024] block of
all four outputs.  MLPs are computed on-device in transposed layout
(features on the 128 partitions, samples on the free dim).  Layer-1
pre-activations are rank<=2, computed with vector ops on host-broadcast
inputs instead of matmuls.
"""
import sys
if "/opt/trn_rl_repo" not in sys.path:
    sys.path.insert(0, "/opt/trn_rl_repo")

import numpy as np

import concourse.bass as bass
import concourse.mybir as mybir
import concourse.tile as tile
from concourse import bacc
from concourse.bass_utils import run_bass_kernel_spmd

P = 128
B_FULL, E_FULL = 4096, 4096
RB, RE = 2, 4                   # core grid: b split RB ways, e split RE ways
NB, NE = B_FULL // RB, E_FULL // RE   # per-core block: 2048 x 1024
FT = 512                        # free-dim tile (psum bank = 512 f32)
NCB, NCE = NB // FT, NE // FT   # chunks: 4 branch, 2 trunk
N_LAYERS = 4
F32 = mybir.dt.float32
F32R = mybir.dt.float32r
ACTF = mybir.ActivationFunctionType
ALU = mybir.AluOpType

_CACHE = {}


def _build(main_dt="f32"):
    """Build + compile the per-core kernel (same program on all 8 cores)."""
    nc = bacc.Bacc("TRN2", target_bir_lowering=False, debug=False)

    # ---- DRAM I/O ----------------------------------------------------------
    # wpack columns: [bW2 | bW3 | bW4 | tW2 | tW3 | tW4]  (each [128,128])
    # vpack columns: bB1..bB4, tB1..tB4, bW1c, wlin, tW1c(2)  -> 12 cols
    dt_t = nc.dram_tensor("dt_t", [1, NB], F32, kind="ExternalInput")
    co_t = nc.dram_tensor("co_t", [2, NE], F32, kind="ExternalInput")
    bW1 = nc.dram_tensor("bW1", [1, P], F32, kind="ExternalInput")
    tW1 = nc.dram_tensor("tW1", [2, P], F32, kind="ExternalInput")
    wpack = nc.dram_tensor("wpack", [P, 6 * P], F32, kind="ExternalInput")
    vpack = nc.dram_tensor("vpack", [P, 12], F32, kind="ExternalInput")
    outs = {
        name: nc.dram_tensor(name, [NB, NE], F32, kind="ExternalOutput")
        for name in ("U", "DX", "DY", "DMU")
    }

    mm_dt = F32 if main_dt == "f32" else F32R

    with tile.TileContext(nc) as tc:
        with (
            tc.tile_pool(name="const", bufs=1) as const,
            tc.tile_pool(name="basis", bufs=1) as basis,
            tc.tile_pool(name="work", bufs=6) as work,
            tc.tile_pool(name="stage", bufs=8) as stage,
            tc.tile_pool(name="ps", bufs=8, space="PSUM") as psp,
        ):
            # ---- load weights / inputs to SBUF (few, packed DMAs) ----------
            def load(name, src, shape):
                t = const.tile(shape, F32, name=name, tag=name)
                nc.sync.dma_start(t[:], src.ap())
                return t

            vp_sb = load("vp_sb", vpack, [P, 12])
            co_sb = load("co_sb", co_t, [2, NE])
            tW1_sb = load("tW1_sb", tW1, [2, P])
            dt_sb = const.tile([1, NB], F32, name="dt_sb", tag="dt_sb")
            nc.scalar.dma_start(dt_sb[:], dt_t.ap())
            bW1_sb = const.tile([1, P], F32, name="bW1_sb", tag="bW1_sb")
            nc.scalar.dma_start(bW1_sb[:], bW1.ap())
            wp_sb = load("wp_sb", wpack, [P, 6 * P])

            bW_sb = [None] + [wp_sb[:, i * P:(i + 1) * P] for i in range(3)]
            tW_sb = [None] + [wp_sb[:, (3 + i) * P:(4 + i) * P]
                              for i in range(3)]
            bB_sb = [vp_sb[:, i:i + 1] for i in range(4)]
            tB_sb = [vp_sb[:, 4 + i:5 + i] for i in range(4)]
            bW1c_sb = vp_sb[:, 8:9]
            wlin_sb = vp_sb[:, 9:10]
            tW1xc_sb = vp_sb[:, 10:11]
            tW1yc_sb = vp_sb[:, 11:12]

            # ---- per-chunk basis tiles (transposed layout) -----------------
            def chunk_tiles(prefix, n):
                return [basis.tile([P, FT], mm_dt, name=f"{prefix}{i}",
                                   tag=f"{prefix}{i}") for i in range(n)]

            A_c = chunk_tiles("A", NCB)
            Ap_c = chunk_tiles("Ap", NCB)
            Tb_c = chunk_tiles("Tb", NCE)
            Tx_c = chunk_tiles("Tx", NCE)
            Ty_c = chunk_tiles("Ty", NCE)

            # ---- one MLP+JVP layer step (emitted in waves) -----------------
            class Chain:
                def __init__(self, ci, pre1_fn, n_tangents, Ws, Bs, seed_cols,
                             finals, w=FT):
                    self.ci = ci
                    self.pre1_fn = pre1_fn
                    self.n_tangents = n_tangents
                    self.Ws, self.Bs = Ws, Bs
                    self.seed_cols = seed_cols
                    self.finals = finals
                    self.w = w
                    self.y = None
                    self.ts = [None] * n_tangents

            def emit_chain_layer(ch, li):
                ci, w = ch.ci, ch.w
                pre1_fn, n_tangents = ch.pre1_fn, ch.n_tangents
                Ws, Bs, seed_cols, finals = ch.Ws, ch.Bs, ch.seed_cols, ch.finals
                y, ts = ch.y, ch.ts
                if True:
                    # forward pre-activation
                    if li == 0:
                        pre = psp.tile([P, w], F32, name=f"p1_{ci}", tag="ps")
                        pre1_fn(pre)
                        pre_ap = pre[:]
                    else:
                        pre = psp.tile([P, w], F32, name=f"pre{ci}_{li}",
                                       tag="ps")
                        nc.tensor.matmul(pre[:], Ws[li], y[:],
                                         start=True, stop=True)
                        pre_ap = pre[:]
                    # tangent pre-products (layers >= 1)
                    tps = []
                    if li > 0:
                        for k in range(n_tangents):
                            tp = psp.tile([P, w], F32,
                                          name=f"tp{ci}_{li}_{k}", tag="ps")
                            nc.tensor.matmul(tp[:], Ws[li], ts[k][:],
                                             start=True, stop=True)
                            tps.append(tp)
                    # y = tanh(pre + b); on the last layer with no final
                    # scale, write straight into the destination tile
                    is_last_fwd_direct = (li == N_LAYERS - 1
                                          and finals[0][1] is None)
                    if is_last_fwd_direct:
                        y_new = None
                        y_ap = finals[0][0]
                    else:
                        y_new = work.tile([P, w], F32, name=f"y{ci}_{li}",
                                          tag="y")
                        y_ap = y_new[:]
                    nc.scalar.activation(y_ap, pre_ap, ACTF.Tanh, bias=Bs[li])
                    # d = 1 - y^2   (square on ACT, affine on DVE)
                    sq = work.tile([P, w], F32, name=f"sq{ci}_{li}", tag="sq")
                    nc.scalar.activation(sq[:], y_ap, ACTF.Square)
                    d = work.tile([P, w], F32, name=f"d{ci}_{li}", tag="d")
                    nc.vector.tensor_scalar(d[:], sq[:], -1.0, 1.0,
                                            ALU.mult, ALU.add)
                    # tangent updates
                    is_last = li == N_LAYERS - 1
                    t_new = []
                    for k in range(n_tangents):
                        if is_last:
                            tk, _scale = finals[1 + k]
                        else:
                            tk_t = work.tile([P, w], F32,
                                             name=f"t{ci}_{li}_{k}", tag=f"t{k}")
                            tk = tk_t[:]
                        if li == 0:
                            # tangent seed: d * seed_col_k
                            nc.vector.tensor_scalar(tk, d[:], seed_cols[k],
                                                    None, ALU.mult)
                        else:
                            nc.vector.tensor_mul(tk, tps[k][:], d[:])
                            if is_last and finals[1 + k][1] is not None:
                                nc.vector.tensor_scalar(tk, tk,
                                                        finals[1 + k][1], None,
                                                        ALU.mult)
                        if not is_last:
                            t_new.append(tk_t)
                    ts = t_new
                    if is_last:
                        dst, scale = finals[0]
                        if scale is not None:
                            nc.vector.tensor_scalar(dst, y_ap, scale,
                                                    None, ALU.mult)
                    y = y_new
                ch.y, ch.ts = y, ts

            # trunk chunks: pre1 = tW1.T @ coords_t  (K=2 matmul)
            def trunk_chain(ci):
                sl = slice(ci * FT, (ci + 1) * FT)

                def trunk_pre1(dst, sl=sl):
                    nc.tensor.matmul(dst[:], tW1_sb[:], co_sb[:, sl],
                                     start=True, stop=True)

                return Chain(f"t{ci}", trunk_pre1, 2, tW_sb, tB_sb,
                             [tW1xc_sb, tW1yc_sb],
                             [(Tb_c[ci][:], wlin_sb), (Tx_c[ci][:], wlin_sb),
                              (Ty_c[ci][:], wlin_sb)])

            # branch chunks: pre1 = bW1.T @ dt_t  (K=1 matmul)
            def branch_chain(ci):
                sl = slice(ci * FT, (ci + 1) * FT)

                def branch_pre1(dst, sl=sl):
                    nc.tensor.matmul(dst[:], bW1_sb[:], dt_sb[:, sl],
                                     start=True, stop=True)

                return Chain(f"b{ci}", branch_pre1, 1, bW_sb, bB_sb, [bW1c_sb],
                             [(A_c[ci][:], None), (Ap_c[ci][:], None)])

            # ---- main loop emitters: 4 outputs, [128 x 512] psum tiles -----
            rhs_for = {"U": Tb_c, "DX": Tx_c, "DY": Ty_c, "DMU": Tb_c}
            lhs_for = {"U": A_c, "DX": A_c, "DY": A_c, "DMU": Ap_c}
            order = ["U", "DX", "DY", "DMU"]
            evict_state = [0]

            def emit_main_bt(bt):
                bsl = slice((bt % 4) * P, (bt % 4 + 1) * P)
                lhs_ci = bt // 4
                for o in order:
                    lhsT = lhs_for[o][lhs_ci]
                    rhs = rhs_for[o]
                    st = stage.tile([P, NE], F32, name=f"st_{o}_{bt}", tag="st")
                    for nt in range(NCE):
                        ps_t = psp.tile([P, FT], F32,
                                        name=f"mm_{o}_{bt}_{nt}", tag="ps")
                        nc.tensor.matmul(ps_t[:], lhsT[:, bsl], rhs[nt][:],
                                         start=True, stop=True)
                        dstv = st[:, nt * FT:(nt + 1) * FT]
                        if evict_state[0] % 2 == 0:
                            nc.scalar.copy(dstv, ps_t[:])
                        else:
                            nc.vector.tensor_copy(dstv, ps_t[:])
                        evict_state[0] += 1
                        nc.sync.dma_start(
                            outs[o].ap()[bt * P:(bt + 1) * P,
                                         nt * FT:(nt + 1) * FT], dstv)

            # layer-wave emission: all chunks advance one layer per wave so
            # the in-order PE stream always has ready matmuls from other
            # chunks while one chunk's elementwise chain completes.  The
            # final branch layers are interleaved with the main tiles they
            # unblock.
            t_chains = [trunk_chain(ci) for ci in range(NCE)]
            b_chains = [branch_chain(ci) for ci in range(NCB)]
            chains = t_chains + b_chains
            for li in range(N_LAYERS - 1):
                for ch in chains:
                    emit_chain_layer(ch, li)
            for ch in t_chains:
                emit_chain_layer(ch, N_LAYERS - 1)
            for ci in range(NCB):
                emit_chain_layer(b_chains[ci], N_LAYERS - 1)
                for bt in range(4 * ci, 4 * ci + 4):
                    emit_main_bt(bt)

    nc.compile()
    return nc


def _get_nc(main_dt):
    if main_dt not in _CACHE:
        _CACHE[main_dt] = _build(main_dt)
    return _CACHE[main_dt]


def kernel(DT, coords, branch_Ws, branch_bs, trunk_Ws, trunk_bs, w_lin,
           _main_dt="f32", _want_results=False):
    DT = np.asarray(DT, np.float32)
    coords = np.asarray(coords, np.float32)
    nc = _get_nc(_main_dt)

    bWs = [np.asarray(w, np.float32) for w in branch_Ws]
    tWs = [np.asarray(w, np.float32) for w in trunk_Ws]
    bbs = [np.asarray(b, np.float32).reshape(P, 1) for b in branch_bs]
    tbs = [np.asarray(b, np.float32).reshape(P, 1) for b in trunk_bs]
    shared = {
        "bW1": np.ascontiguousarray(bWs[0]),
        "tW1": np.ascontiguousarray(tWs[0]),
        "wpack": np.ascontiguousarray(
            np.concatenate(bWs[1:] + tWs[1:], axis=1)),
        "vpack": np.ascontiguousarray(np.concatenate(
            bbs + tbs + [bWs[0].T, np.asarray(w_lin, np.float32).reshape(P, 1),
                         tWs[0].T], axis=1)),
    }

    in_maps = []
    for c in range(RB * RE):
        rb, re = c // RE, c % RE
        m = dict(shared)
        m["dt_t"] = np.ascontiguousarray(DT[rb * NB:(rb + 1) * NB, :].T)
        m["co_t"] = np.ascontiguousarray(coords[re * NE:(re + 1) * NE, :].T)
        in_maps.append(m)

    res = run_bass_kernel_spmd(nc, in_maps, core_ids=list(range(RB * RE)))

    full = {k: np.empty((B_FULL, E_FULL), np.float32)
            for k in ("U", "DX", "DY", "DMU")}
    for c in range(RB * RE):
        rb, re = c // RE, c % RE
        for k in full:
            full[k][rb * NB:(rb + 1) * NB, re * NE:(re + 1) * NE] = \
                res.results[c][k]
    out = tuple(full[k].reshape(B_FULL, E_FULL, 1)
                for k in ("U", "DX", "DY", "DMU"))
    if _want_results:
        return out, res
    return out


# revision 29
# speedup vs baseline: 1.0124x; 1.0124x over previous
"""DeepONet forward + JVPs on 8 Trainium2 NeuronCores (Bass/Tile).

Math (see reference):
  branch MLP (4x tanh layers, in_dim 1)  -> basis_br, dbr_mu   [b, 128]
  trunk  MLP (4x tanh layers, in_dim 2)  -> basis_tr, dtr_x, dtr_y  [e, 128]
  A  = basis_br * w_lin.T ; Ap = dbr_mu * w_lin.T
  U     = A  @ basis_tr.T      dU_x = A @ dtr_x.T
  dU_y  = A  @ dtr_y.T         dU_mu = Ap @ basis_tr.T
  outputs each [b, e, 1] f32

Sharding: 2x4 grid over (b, e): each core computes a [2048, 1024] block of
all four outputs.  MLPs are computed on-device in transposed layout
(features on the 128 partitions, samples on the free dim).  Layer-1
pre-activations are rank<=2, computed with vector ops on host-broadcast
inputs instead of matmuls.
"""
import sys
if "/opt/trn_rl_repo" not in sys.path:
    sys.path.insert(0, "/opt/trn_rl_repo")

import numpy as np

import concourse.bass as bass
import concourse.mybir as mybir
import concourse.tile as tile
from concourse import bacc
from concourse.bass_utils import run_bass_kernel_spmd

P = 128
B_FULL, E_FULL = 4096, 4096
RB, RE = 2, 4                   # core grid: b split RB ways, e split RE ways
NB, NE = B_FULL // RB, E_FULL // RE   # per-core block: 2048 x 1024
FT = 512                        # free-dim tile (psum bank = 512 f32)
NCB, NCE = NB // FT, NE // FT   # chunks: 4 branch, 2 trunk
N_LAYERS = 4
F32 = mybir.dt.float32
F32R = mybir.dt.float32r
ACTF = mybir.ActivationFunctionType
ALU = mybir.AluOpType

_CACHE = {}


def _build(main_dt="f32"):
    """Build + compile the per-core kernel (same program on all 8 cores)."""
    nc = bacc.Bacc("TRN2", target_bir_lowering=False, debug=False)

    # ---- DRAM I/O ----------------------------------------------------------
    # wpack columns: [bW2 | bW3 | bW4 | tW2 | tW3 | tW4]  (each [128,128])
    # vpack columns: bB1..bB4, tB1..tB4, bW1c, wlin, tW1c(2)  -> 12 cols
    dtb = nc.dram_tensor("dtb", [P, NB], F32, kind="ExternalInput")   # bcast DT
    cxb = nc.dram_tensor("cxb", [P, NE], F32, kind="ExternalInput")   # bcast x
    cyb = nc.dram_tensor("cyb", [P, NE], F32, kind="ExternalInput")   # bcast y
    wpack = nc.dram_tensor("wpack", [P, 6 * P], F32, kind="ExternalInput")
    vpack = nc.dram_tensor("vpack", [P, 12], F32, kind="ExternalInput")
    outs = {
        name: nc.dram_tensor(name, [NB, NE], F32, kind="ExternalOutput")
        for name in ("U", "DX", "DY", "DMU")
    }

    mm_dt = F32 if main_dt == "f32" else F32R

    with tile.TileContext(nc) as tc:
        with (
            tc.tile_pool(name="const", bufs=1) as const,
            tc.tile_pool(name="basis", bufs=1) as basis,
            tc.tile_pool(name="work", bufs=6) as work,
            tc.tile_pool(name="stage", bufs=8) as stage,
            tc.tile_pool(name="ps", bufs=8, space="PSUM") as psp,
        ):
            # ---- load weights / inputs to SBUF (few, packed DMAs) ----------
            def load(name, src, shape):
                t = const.tile(shape, F32, name=name, tag=name)
                nc.sync.dma_start(t[:], src.ap())
                return t

            vp_sb = load("vp_sb", vpack, [P, 12])
            cxb_sb = load("cxb_sb", cxb, [P, NE])
            cyb_sb = const.tile([P, NE], F32, name="cyb_sb", tag="cyb_sb")
            nc.scalar.dma_start(cyb_sb[:], cyb.ap())
            wp_sb = load("wp_sb", wpack, [P, 6 * P])
            dtb_sb = const.tile([P, NB], F32, name="dtb_sb", tag="dtb_sb")
            nc.scalar.dma_start(dtb_sb[:], dtb.ap())

            bW_sb = [None] + [wp_sb[:, i * P:(i + 1) * P] for i in range(3)]
            tW_sb = [None] + [wp_sb[:, (3 + i) * P:(4 + i) * P]
                              for i in range(3)]
            bB_sb = [vp_sb[:, i:i + 1] for i in range(4)]
            tB_sb = [vp_sb[:, 4 + i:5 + i] for i in range(4)]
            bW1c_sb = vp_sb[:, 8:9]
            wlin_sb = vp_sb[:, 9:10]
            tW1xc_sb = vp_sb[:, 10:11]
            tW1yc_sb = vp_sb[:, 11:12]

            # ---- per-chunk basis tiles (transposed layout) -----------------
            def chunk_tiles(prefix, n):
                return [basis.tile([P, FT], mm_dt, name=f"{prefix}{i}",
                                   tag=f"{prefix}{i}") for i in range(n)]

            A_c = chunk_tiles("A", NCB)
            Ap_c = chunk_tiles("Ap", NCB)
            Tb_c = chunk_tiles("Tb", NCE)
            Tx_c = chunk_tiles("Tx", NCE)
            Ty_c = chunk_tiles("Ty", NCE)

            # ---- one MLP+JVP layer step (emitted in waves) -----------------
            class Chain:
                def __init__(self, ci, pre1_fn, n_tangents, Ws, Bs, seed_cols,
                             finals, w=FT):
                    self.ci = ci
                    self.pre1_fn = pre1_fn
                    self.n_tangents = n_tangents
                    self.Ws, self.Bs = Ws, Bs
                    self.seed_cols = seed_cols
                    self.finals = finals
                    self.w = w
                    self.y = None
                    self.ts = [None] * n_tangents

            def emit_chain_layer(ch, li):
                ci, w = ch.ci, ch.w
                pre1_fn, n_tangents = ch.pre1_fn, ch.n_tangents
                Ws, Bs, seed_cols, finals = ch.Ws, ch.Bs, ch.seed_cols, ch.finals
                y, ts = ch.y, ch.ts
                if True:
                    # forward pre-activation
                    if li == 0:
                        pre_sb = work.tile([P, w], F32, name=f"p1_{ci}",
                                           tag="pre1")
                        pre1_fn(pre_sb)
                        pre_ap = pre_sb[:]
                    else:
                        pre = psp.tile([P, w], F32, name=f"pre{ci}_{li}",
                                       tag="ps")
                        nc.tensor.matmul(pre[:], Ws[li], y[:],
                                         start=True, stop=True)
                        pre_ap = pre[:]
                    # tangent pre-products (layers >= 1)
                    tps = []
                    if li > 0:
                        for k in range(n_tangents):
                            tp = psp.tile([P, w], F32,
                                          name=f"tp{ci}_{li}_{k}", tag="ps")
                            nc.tensor.matmul(tp[:], Ws[li], ts[k][:],
                                             start=True, stop=True)
                            tps.append(tp)
                    # y = tanh(pre + b); on the last layer with no final
                    # scale, write straight into the destination tile
                    is_last_fwd_direct = (li == N_LAYERS - 1
                                          and finals[0][1] is None)
                    if is_last_fwd_direct:
                        y_new = None
                        y_ap = finals[0][0]
                    else:
                        y_new = work.tile([P, w], F32, name=f"y{ci}_{li}",
                                          tag="y")
                        y_ap = y_new[:]
                    nc.scalar.activation(y_ap, pre_ap, ACTF.Tanh, bias=Bs[li])
                    # d = 1 - y^2   (square on ACT, affine on DVE)
                    sq = work.tile([P, w], F32, name=f"sq{ci}_{li}", tag="sq")
                    nc.scalar.activation(sq[:], y_ap, ACTF.Square)
                    d = work.tile([P, w], F32, name=f"d{ci}_{li}", tag="d")
                    nc.vector.tensor_scalar(d[:], sq[:], -1.0, 1.0,
                                            ALU.mult, ALU.add)
                    # tangent updates
                    is_last = li == N_LAYERS - 1
                    t_new = []
                    for k in range(n_tangents):
                        if is_last:
                            tk, _scale = finals[1 + k]
                        else:
                            tk_t = work.tile([P, w], F32,
                                             name=f"t{ci}_{li}_{k}", tag=f"t{k}")
                            tk = tk_t[:]
                        if li == 0:
                            # tangent seed: d * seed_col_k
                            nc.vector.tensor_scalar(tk, d[:], seed_cols[k],
                                                    None, ALU.mult)
                        else:
                            nc.vector.tensor_mul(tk, tps[k][:], d[:])
                            if is_last and finals[1 + k][1] is not None:
                                nc.vector.tensor_scalar(tk, tk,
                                                        finals[1 + k][1], None,
                                                        ALU.mult)
                        if not is_last:
                            t_new.append(tk_t)
                    ts = t_new
                    if is_last:
                        dst, scale = finals[0]
                        if scale is not None:
                            nc.vector.tensor_scalar(dst, y_ap, scale,
                                                    None, ALU.mult)
                    y = y_new
                ch.y, ch.ts = y, ts

            # trunk chunks: pre1 = w1x*X + (w1y*Y + b1)
            def trunk_chain(ci):
                sl = slice(ci * FT, (ci + 1) * FT)

                def trunk_pre1(dst, sl=sl):
                    tmp = work.tile([P, FT], F32, name="tp1", tag="tp1")
                    nc.scalar.activation(tmp[:], cyb_sb[:, sl], ACTF.Identity,
                                         bias=tB_sb[0], scale=tW1yc_sb)
                    nc.vector.tensor_scalar(dst[:], cxb_sb[:, sl], tW1xc_sb,
                                            None, ALU.mult)
                    nc.vector.tensor_add(dst[:], dst[:], tmp[:])

                return Chain(f"t{ci}", trunk_pre1, 2, tW_sb, tB_sb,
                             [tW1xc_sb, tW1yc_sb],
                             [(Tb_c[ci][:], wlin_sb), (Tx_c[ci][:], wlin_sb),
                              (Ty_c[ci][:], wlin_sb)])

            # branch chunks: pre1 = w1 * DT + b1
            def branch_chain(ci):
                sl = slice(ci * FT, (ci + 1) * FT)

                def branch_pre1(dst, sl=sl):
                    nc.scalar.activation(dst[:], dtb_sb[:, sl], ACTF.Identity,
                                         bias=bB_sb[0], scale=bW1c_sb)

                return Chain(f"b{ci}", branch_pre1, 1, bW_sb, bB_sb, [bW1c_sb],
                             [(A_c[ci][:], None), (Ap_c[ci][:], None)])

            # ---- main loop emitters: 4 outputs, [128 x 512] psum tiles -----
            rhs_for = {"U": Tb_c, "DX": Tx_c, "DY": Ty_c, "DMU": Tb_c}
            lhs_for = {"U": A_c, "DX": A_c, "DY": A_c, "DMU": Ap_c}
            order = ["U", "DX", "DY", "DMU"]
            evict_state = [0]

            def emit_main_bt(bt):
                bsl = slice((bt % 4) * P, (bt % 4 + 1) * P)
                lhs_ci = bt // 4
                for o in order:
                    lhsT = lhs_for[o][lhs_ci]
                    rhs = rhs_for[o]
                    st = stage.tile([P, NE], F32, name=f"st_{o}_{bt}", tag="st")
                    for nt in range(NCE):
                        ps_t = psp.tile([P, FT], F32,
                                        name=f"mm_{o}_{bt}_{nt}", tag="ps")
                        nc.tensor.matmul(ps_t[:], lhsT[:, bsl], rhs[nt][:],
                                         start=True, stop=True)
                        dstv = st[:, nt * FT:(nt + 1) * FT]
                        if evict_state[0] % 2 == 0:
                            nc.scalar.copy(dstv, ps_t[:])
                        else:
                            nc.vector.tensor_copy(dstv, ps_t[:])
                        evict_state[0] += 1
                        nc.sync.dma_start(
                            outs[o].ap()[bt * P:(bt + 1) * P,
                                         nt * FT:(nt + 1) * FT], dstv)

            # layer-wave emission: all chunks advance one layer per wave so
            # the in-order PE stream always has ready matmuls from other
            # chunks while one chunk's elementwise chain completes.  The
            # final branch layers are interleaved with the main tiles they
            # unblock.
            t_chains = [trunk_chain(ci) for ci in range(NCE)]
            b_chains = [branch_chain(ci) for ci in range(NCB)]
            chains = t_chains + b_chains
            for li in range(N_LAYERS - 1):
                for ch in chains:
                    emit_chain_layer(ch, li)
            for ch in t_chains:
                emit_chain_layer(ch, N_LAYERS - 1)
            for ci in range(NCB):
                emit_chain_layer(b_chains[ci], N_LAYERS - 1)
                for bt in range(4 * ci, 4 * ci + 4):
                    emit_main_bt(bt)

    nc.compile()
    return nc


def _get_nc(main_dt):
    if main_dt not in _CACHE:
        _CACHE[main_dt] = _build(main_dt)
    return _CACHE[main_dt]


def kernel(DT, coords, branch_Ws, branch_bs, trunk_Ws, trunk_bs, w_lin,
           _main_dt="f32", _want_results=False):
    DT = np.asarray(DT, np.float32)
    coords = np.asarray(coords, np.float32)
    nc = _get_nc(_main_dt)

    bWs = [np.asarray(w, np.float32) for w in branch_Ws]
    tWs = [np.asarray(w, np.float32) for w in trunk_Ws]
    bbs = [np.asarray(b, np.float32).reshape(P, 1) for b in branch_bs]
    tbs = [np.asarray(b, np.float32).reshape(P, 1) for b in trunk_bs]
    shared = {
        "wpack": np.ascontiguousarray(
            np.concatenate(bWs[1:] + tWs[1:], axis=1)),
        "vpack": np.ascontiguousarray(np.concatenate(
            bbs + tbs + [bWs[0].T, np.asarray(w_lin, np.float32).reshape(P, 1),
                         tWs[0].T], axis=1)),
    }

    in_maps = []
    for c in range(RB * RE):
        rb, re = c // RE, c % RE
        m = dict(shared)
        m["dtb"] = np.ascontiguousarray(np.broadcast_to(
            DT[rb * NB:(rb + 1) * NB, :].T, (P, NB)))
        csl = coords[re * NE:(re + 1) * NE, :]
        m["cxb"] = np.ascontiguousarray(np.broadcast_to(csl[:, 0], (P, NE)))
        m["cyb"] = np.ascontiguousarray(np.broadcast_to(csl[:, 1], (P, NE)))
        in_maps.append(m)

    res = run_bass_kernel_spmd(nc, in_maps, core_ids=list(range(RB * RE)))

    full = {k: np.empty((B_FULL, E_FULL), np.float32)
            for k in ("U", "DX", "DY", "DMU")}
    for c in range(RB * RE):
        rb, re = c // RE, c % RE
        for k in full:
            full[k][rb * NB:(rb + 1) * NB, re * NE:(re + 1) * NE] = \
                res.results[c][k]
    out = tuple(full[k].reshape(B_FULL, E_FULL, 1)
                for k in ("U", "DX", "DY", "DMU"))
    if _want_results:
        return out, res
    return out


# revision 30
# speedup vs baseline: 1.0131x; 1.0007x over previous
"""DeepONet forward + JVPs on 8 Trainium2 NeuronCores (Bass/Tile).

Math (see reference):
  branch MLP (4x tanh layers, in_dim 1)  -> basis_br, dbr_mu   [b, 128]
  trunk  MLP (4x tanh layers, in_dim 2)  -> basis_tr, dtr_x, dtr_y  [e, 128]
  A  = basis_br * w_lin.T ; Ap = dbr_mu * w_lin.T
  U     = A  @ basis_tr.T      dU_x = A @ dtr_x.T
  dU_y  = A  @ dtr_y.T         dU_mu = Ap @ basis_tr.T
  outputs each [b, e, 1] f32

Sharding: 2x4 grid over (b, e): each core computes a [2048, 1024] block of
all four outputs.  MLPs are computed on-device in transposed layout
(features on the 128 partitions, samples on the free dim).  Layer-1
pre-activations are rank<=2, computed with vector ops on host-broadcast
inputs instead of matmuls.
"""
import sys
if "/opt/trn_rl_repo" not in sys.path:
    sys.path.insert(0, "/opt/trn_rl_repo")

import numpy as np

import concourse.bass as bass
import concourse.mybir as mybir
import concourse.tile as tile
from concourse import bacc
from concourse.bass_utils import run_bass_kernel_spmd

P = 128
B_FULL, E_FULL = 4096, 4096
RB, RE = 2, 4                   # core grid: b split RB ways, e split RE ways
NB, NE = B_FULL // RB, E_FULL // RE   # per-core block: 2048 x 1024
FT = 512                        # free-dim tile (psum bank = 512 f32)
NCB, NCE = NB // FT, NE // FT   # chunks: 4 branch, 2 trunk
N_LAYERS = 4
F32 = mybir.dt.float32
F32R = mybir.dt.float32r
ACTF = mybir.ActivationFunctionType
ALU = mybir.AluOpType

_CACHE = {}


def _build(main_dt="f32"):
    """Build + compile the per-core kernel (same program on all 8 cores)."""
    nc = bacc.Bacc("TRN2", target_bir_lowering=False, debug=False)

    # ---- DRAM I/O ----------------------------------------------------------
    # wpack columns: [bW2 | bW3 | bW4 | tW2 | tW3 | tW4]  (each [128,128])
    # vpack columns: bB1..bB4, tB1..tB4, bW1c, wlin, tW1c(2)  -> 12 cols
    dtb = nc.dram_tensor("dtb", [P, NB], F32, kind="ExternalInput")   # bcast DT
    cxb = nc.dram_tensor("cxb", [P, NE], F32, kind="ExternalInput")   # bcast x
    cyb = nc.dram_tensor("cyb", [P, NE], F32, kind="ExternalInput")   # bcast y
    wpack = nc.dram_tensor("wpack", [P, 6 * P], F32, kind="ExternalInput")
    vpack = nc.dram_tensor("vpack", [P, 12], F32, kind="ExternalInput")
    outs = {
        name: nc.dram_tensor(name, [NB, NE], F32, kind="ExternalOutput")
        for name in ("U", "DX", "DY", "DMU")
    }

    mm_dt = F32 if main_dt == "f32" else F32R

    with tile.TileContext(nc) as tc:
        with (
            tc.tile_pool(name="const", bufs=1) as const,
            tc.tile_pool(name="basis", bufs=1) as basis,
            tc.tile_pool(name="work", bufs=6) as work,
            tc.tile_pool(name="stage", bufs=8) as stage,
            tc.tile_pool(name="ps", bufs=8, space="PSUM") as psp,
        ):
            # ---- load weights / inputs to SBUF (few, packed DMAs) ----------
            def load(name, src, shape):
                t = const.tile(shape, F32, name=name, tag=name)
                nc.sync.dma_start(t[:], src.ap())
                return t

            vp_sb = load("vp_sb", vpack, [P, 12])
            cxb_sb = const.tile([P, NE], F32, name="cxb_sb", tag="cxb_sb")
            cyb_sb = const.tile([P, NE], F32, name="cyb_sb", tag="cyb_sb")
            for ci in range(NCE):
                sl = slice(ci * FT, (ci + 1) * FT)
                nc.sync.dma_start(cxb_sb[:, sl], cxb.ap()[:, sl])
                nc.scalar.dma_start(cyb_sb[:, sl], cyb.ap()[:, sl])
            wp_sb = load("wp_sb", wpack, [P, 6 * P])
            dtb_sb = const.tile([P, NB], F32, name="dtb_sb", tag="dtb_sb")
            nc.scalar.dma_start(dtb_sb[:], dtb.ap())

            bW_sb = [None] + [wp_sb[:, i * P:(i + 1) * P] for i in range(3)]
            tW_sb = [None] + [wp_sb[:, (3 + i) * P:(4 + i) * P]
                              for i in range(3)]
            bB_sb = [vp_sb[:, i:i + 1] for i in range(4)]
            tB_sb = [vp_sb[:, 4 + i:5 + i] for i in range(4)]
            bW1c_sb = vp_sb[:, 8:9]
            wlin_sb = vp_sb[:, 9:10]
            tW1xc_sb = vp_sb[:, 10:11]
            tW1yc_sb = vp_sb[:, 11:12]

            # ---- per-chunk basis tiles (transposed layout) -----------------
            def chunk_tiles(prefix, n):
                return [basis.tile([P, FT], mm_dt, name=f"{prefix}{i}",
                                   tag=f"{prefix}{i}") for i in range(n)]

            A_c = chunk_tiles("A", NCB)
            Ap_c = chunk_tiles("Ap", NCB)
            Tb_c = chunk_tiles("Tb", NCE)
            Tx_c = chunk_tiles("Tx", NCE)
            Ty_c = chunk_tiles("Ty", NCE)

            # ---- one MLP+JVP layer step (emitted in waves) -----------------
            class Chain:
                def __init__(self, ci, pre1_fn, n_tangents, Ws, Bs, seed_cols,
                             finals, w=FT):
                    self.ci = ci
                    self.pre1_fn = pre1_fn
                    self.n_tangents = n_tangents
                    self.Ws, self.Bs = Ws, Bs
                    self.seed_cols = seed_cols
                    self.finals = finals
                    self.w = w
                    self.y = None
                    self.ts = [None] * n_tangents

            def emit_chain_layer(ch, li):
                ci, w = ch.ci, ch.w
                pre1_fn, n_tangents = ch.pre1_fn, ch.n_tangents
                Ws, Bs, seed_cols, finals = ch.Ws, ch.Bs, ch.seed_cols, ch.finals
                y, ts = ch.y, ch.ts
                if True:
                    # forward pre-activation
                    if li == 0:
                        pre_sb = work.tile([P, w], F32, name=f"p1_{ci}",
                                           tag="pre1")
                        pre1_fn(pre_sb)
                        pre_ap = pre_sb[:]
                    else:
                        pre = psp.tile([P, w], F32, name=f"pre{ci}_{li}",
                                       tag="ps")
                        nc.tensor.matmul(pre[:], Ws[li], y[:],
                                         start=True, stop=True)
                        pre_ap = pre[:]
                    # tangent pre-products (layers >= 1)
                    tps = []
                    if li > 0:
                        for k in range(n_tangents):
                            tp = psp.tile([P, w], F32,
                                          name=f"tp{ci}_{li}_{k}", tag="ps")
                            nc.tensor.matmul(tp[:], Ws[li], ts[k][:],
                                             start=True, stop=True)
                            tps.append(tp)
                    # y = tanh(pre + b); on the last layer with no final
                    # scale, write straight into the destination tile
                    is_last_fwd_direct = (li == N_LAYERS - 1
                                          and finals[0][1] is None)
                    if is_last_fwd_direct:
                        y_new = None
                        y_ap = finals[0][0]
                    else:
                        y_new = work.tile([P, w], F32, name=f"y{ci}_{li}",
                                          tag="y")
                        y_ap = y_new[:]
                    nc.scalar.activation(y_ap, pre_ap, ACTF.Tanh, bias=Bs[li])
                    # d = 1 - y^2   (square on ACT, affine on DVE)
                    sq = work.tile([P, w], F32, name=f"sq{ci}_{li}", tag="sq")
                    nc.scalar.activation(sq[:], y_ap, ACTF.Square)
                    d = work.tile([P, w], F32, name=f"d{ci}_{li}", tag="d")
                    nc.vector.tensor_scalar(d[:], sq[:], -1.0, 1.0,
                                            ALU.mult, ALU.add)
                    # tangent updates
                    is_last = li == N_LAYERS - 1
                    t_new = []
                    for k in range(n_tangents):
                        if is_last:
                            tk, _scale = finals[1 + k]
                        else:
                            tk_t = work.tile([P, w], F32,
                                             name=f"t{ci}_{li}_{k}", tag=f"t{k}")
                            tk = tk_t[:]
                        if li == 0:
                            # tangent seed: d * seed_col_k
                            nc.vector.tensor_scalar(tk, d[:], seed_cols[k],
                                                    None, ALU.mult)
                        else:
                            nc.vector.tensor_mul(tk, tps[k][:], d[:])
                            if is_last and finals[1 + k][1] is not None:
                                nc.vector.tensor_scalar(tk, tk,
                                                        finals[1 + k][1], None,
                                                        ALU.mult)
                        if not is_last:
                            t_new.append(tk_t)
                    ts = t_new
                    if is_last:
                        dst, scale = finals[0]
                        if scale is not None:
                            nc.vector.tensor_scalar(dst, y_ap, scale,
                                                    None, ALU.mult)
                    y = y_new
                ch.y, ch.ts = y, ts

            # trunk chunks: pre1 = w1x*X + (w1y*Y + b1)
            def trunk_chain(ci):
                sl = slice(ci * FT, (ci + 1) * FT)

                def trunk_pre1(dst, sl=sl):
                    tmp = work.tile([P, FT], F32, name="tp1", tag="tp1")
                    nc.scalar.activation(tmp[:], cyb_sb[:, sl], ACTF.Identity,
                                         bias=tB_sb[0], scale=tW1yc_sb)
                    nc.vector.tensor_scalar(dst[:], cxb_sb[:, sl], tW1xc_sb,
                                            None, ALU.mult)
                    nc.vector.tensor_add(dst[:], dst[:], tmp[:])

                return Chain(f"t{ci}", trunk_pre1, 2, tW_sb, tB_sb,
                             [tW1xc_sb, tW1yc_sb],
                             [(Tb_c[ci][:], wlin_sb), (Tx_c[ci][:], wlin_sb),
                              (Ty_c[ci][:], wlin_sb)])

            # branch chunks: pre1 = w1 * DT + b1
            def branch_chain(ci):
                sl = slice(ci * FT, (ci + 1) * FT)

                def branch_pre1(dst, sl=sl):
                    nc.scalar.activation(dst[:], dtb_sb[:, sl], ACTF.Identity,
                                         bias=bB_sb[0], scale=bW1c_sb)

                return Chain(f"b{ci}", branch_pre1, 1, bW_sb, bB_sb, [bW1c_sb],
                             [(A_c[ci][:], None), (Ap_c[ci][:], None)])

            # ---- main loop emitters: 4 outputs, [128 x 512] psum tiles -----
            rhs_for = {"U": Tb_c, "DX": Tx_c, "DY": Ty_c, "DMU": Tb_c}
            lhs_for = {"U": A_c, "DX": A_c, "DY": A_c, "DMU": Ap_c}
            order = ["U", "DX", "DY", "DMU"]
            evict_state = [0]

            def emit_main_bt(bt):
                bsl = slice((bt % 4) * P, (bt % 4 + 1) * P)
                lhs_ci = bt // 4
                for o in order:
                    lhsT = lhs_for[o][lhs_ci]
                    rhs = rhs_for[o]
                    st = stage.tile([P, NE], F32, name=f"st_{o}_{bt}", tag="st")
                    for nt in range(NCE):
                        ps_t = psp.tile([P, FT], F32,
                                        name=f"mm_{o}_{bt}_{nt}", tag="ps")
                        nc.tensor.matmul(ps_t[:], lhsT[:, bsl], rhs[nt][:],
                                         start=True, stop=True)
                        dstv = st[:, nt * FT:(nt + 1) * FT]
                        if evict_state[0] % 2 == 0:
                            nc.scalar.copy(dstv, ps_t[:])
                        else:
                            nc.vector.tensor_copy(dstv, ps_t[:])
                        evict_state[0] += 1
                        nc.sync.dma_start(
                            outs[o].ap()[bt * P:(bt + 1) * P,
                                         nt * FT:(nt + 1) * FT], dstv)

            # layer-wave emission: all chunks advance one layer per wave so
            # the in-order PE stream always has ready matmuls from other
            # chunks while one chunk's elementwise chain completes.  The
            # final branch layers are interleaved with the main tiles they
            # unblock.
            t_chains = [trunk_chain(ci) for ci in range(NCE)]
            b_chains = [branch_chain(ci) for ci in range(NCB)]
            chains = t_chains + b_chains
            for li in range(N_LAYERS - 1):
                for ch in chains:
                    emit_chain_layer(ch, li)
            for ch in t_chains:
                emit_chain_layer(ch, N_LAYERS - 1)
            for ci in range(NCB):
                emit_chain_layer(b_chains[ci], N_LAYERS - 1)
                for bt in range(4 * ci, 4 * ci + 4):
                    emit_main_bt(bt)

    nc.compile()
    return nc


def _get_nc(main_dt):
    if main_dt not in _CACHE:
        _CACHE[main_dt] = _build(main_dt)
    return _CACHE[main_dt]


def kernel(DT, coords, branch_Ws, branch_bs, trunk_Ws, trunk_bs, w_lin,
           _main_dt="f32", _want_results=False):
    DT = np.asarray(DT, np.float32)
    coords = np.asarray(coords, np.float32)
    nc = _get_nc(_main_dt)

    bWs = [np.asarray(w, np.float32) for w in branch_Ws]
    tWs = [np.asarray(w, np.float32) for w in trunk_Ws]
    bbs = [np.asarray(b, np.float32).reshape(P, 1) for b in branch_bs]
    tbs = [np.asarray(b, np.float32).reshape(P, 1) for b in trunk_bs]
    shared = {
        "wpack": np.ascontiguousarray(
            np.concatenate(bWs[1:] + tWs[1:], axis=1)),
        "vpack": np.ascontiguousarray(np.concatenate(
            bbs + tbs + [bWs[0].T, np.asarray(w_lin, np.float32).reshape(P, 1),
                         tWs[0].T], axis=1)),
    }

    in_maps = []
    for c in range(RB * RE):
        rb, re = c // RE, c % RE
        m = dict(shared)
        m["dtb"] = np.ascontiguousarray(np.broadcast_to(
            DT[rb * NB:(rb + 1) * NB, :].T, (P, NB)))
        csl = coords[re * NE:(re + 1) * NE, :]
        m["cxb"] = np.ascontiguousarray(np.broadcast_to(csl[:, 0], (P, NE)))
        m["cyb"] = np.ascontiguousarray(np.broadcast_to(csl[:, 1], (P, NE)))
        in_maps.append(m)

    res = run_bass_kernel_spmd(nc, in_maps, core_ids=list(range(RB * RE)))

    full = {k: np.empty((B_FULL, E_FULL), np.float32)
            for k in ("U", "DX", "DY", "DMU")}
    for c in range(RB * RE):
        rb, re = c // RE, c % RE
        for k in full:
            full[k][rb * NB:(rb + 1) * NB, re * NE:(re + 1) * NE] = \
                res.results[c][k]
    out = tuple(full[k].reshape(B_FULL, E_FULL, 1)
                for k in ("U", "DX", "DY", "DMU"))
    if _want_results:
        return out, res
    return out


# revision 31
# speedup vs baseline: 1.0391x; 1.0256x over previous
"""DeepONet forward + JVPs on 8 Trainium2 NeuronCores (Bass/Tile).

Math (see reference):
  branch MLP (4x tanh layers, in_dim 1)  -> basis_br, dbr_mu   [b, 128]
  trunk  MLP (4x tanh layers, in_dim 2)  -> basis_tr, dtr_x, dtr_y  [e, 128]
  A  = basis_br * w_lin.T ; Ap = dbr_mu * w_lin.T
  U     = A  @ basis_tr.T      dU_x = A @ dtr_x.T
  dU_y  = A  @ dtr_y.T         dU_mu = Ap @ basis_tr.T
  outputs each [b, e, 1] f32

Sharding: 2x4 grid over (b, e): each core computes a [2048, 1024] block of
all four outputs.  MLPs are computed on-device in transposed layout
(features on the 128 partitions, samples on the free dim).  Layer-1
pre-activations are rank<=2, computed with vector ops on host-broadcast
inputs instead of matmuls.
"""
import sys
if "/opt/trn_rl_repo" not in sys.path:
    sys.path.insert(0, "/opt/trn_rl_repo")

import numpy as np

import concourse.bass as bass
import concourse.mybir as mybir
import concourse.tile as tile
from concourse import bacc
from concourse.bass_utils import run_bass_kernel_spmd

P = 128
B_FULL, E_FULL = 4096, 4096
RB, RE = 2, 4                   # core grid: b split RB ways, e split RE ways
NB, NE = B_FULL // RB, E_FULL // RE   # per-core block: 2048 x 1024
FT = 512                        # free-dim tile (psum bank = 512 f32)
NCB, NCE = NB // FT, NE // FT   # chunks: 4 branch, 2 trunk
N_LAYERS = 4
F32 = mybir.dt.float32
F32R = mybir.dt.float32r
ACTF = mybir.ActivationFunctionType
ALU = mybir.AluOpType

_CACHE = {}


def _build(main_dt="f32"):
    """Build + compile the per-core kernel (same program on all 8 cores)."""
    nc = bacc.Bacc("TRN2", target_bir_lowering=False, debug=False)

    # ---- DRAM I/O ----------------------------------------------------------
    # wpack columns: [bW2 | bW3 | bW4 | tW2 | tW3 | tW4]  (each [128,128])
    # vpack columns: bB1..bB4, tB1..tB4, bW1c, wlin, tW1c(2)  -> 12 cols
    dtb = nc.dram_tensor("dtb", [P, NB], F32, kind="ExternalInput")   # bcast DT
    co_t = nc.dram_tensor("co_t", [2, NE], F32, kind="ExternalInput")
    tW1 = nc.dram_tensor("tW1", [2, P], F32, kind="ExternalInput")
    wpack = nc.dram_tensor("wpack", [P, 6 * P], F32, kind="ExternalInput")
    vpack = nc.dram_tensor("vpack", [P, 12], F32, kind="ExternalInput")
    outs = {
        name: nc.dram_tensor(name, [NB, NE], F32, kind="ExternalOutput")
        for name in ("U", "DX", "DY", "DMU")
    }

    mm_dt = F32 if main_dt == "f32" else F32R

    with tile.TileContext(nc) as tc:
        with (
            tc.tile_pool(name="const", bufs=1) as const,
            tc.tile_pool(name="basis", bufs=1) as basis,
            tc.tile_pool(name="work", bufs=6) as work,
            tc.tile_pool(name="stage", bufs=8) as stage,
            tc.tile_pool(name="ps", bufs=8, space="PSUM") as psp,
        ):
            # ---- load weights / inputs to SBUF (few, packed DMAs) ----------
            def load(name, src, shape):
                t = const.tile(shape, F32, name=name, tag=name)
                nc.sync.dma_start(t[:], src.ap())
                return t

            co_sb = load("co_sb", co_t, [2, NE])
            tW1_sb = load("tW1_sb", tW1, [2, P])
            vp_sb = load("vp_sb", vpack, [P, 12])
            wp_sb = load("wp_sb", wpack, [P, 6 * P])
            dtb_sb = const.tile([P, NB], F32, name="dtb_sb", tag="dtb_sb")
            nc.scalar.dma_start(dtb_sb[:], dtb.ap())

            bW_sb = [None] + [wp_sb[:, i * P:(i + 1) * P] for i in range(3)]
            tW_sb = [None] + [wp_sb[:, (3 + i) * P:(4 + i) * P]
                              for i in range(3)]
            bB_sb = [vp_sb[:, i:i + 1] for i in range(4)]
            tB_sb = [vp_sb[:, 4 + i:5 + i] for i in range(4)]
            bW1c_sb = vp_sb[:, 8:9]
            wlin_sb = vp_sb[:, 9:10]
            tW1xc_sb = vp_sb[:, 10:11]
            tW1yc_sb = vp_sb[:, 11:12]

            # ---- per-chunk basis tiles (transposed layout) -----------------
            def chunk_tiles(prefix, n):
                return [basis.tile([P, FT], mm_dt, name=f"{prefix}{i}",
                                   tag=f"{prefix}{i}") for i in range(n)]

            A_c = chunk_tiles("A", NCB)
            Ap_c = chunk_tiles("Ap", NCB)
            Tb_c = chunk_tiles("Tb", NCE)
            Tx_c = chunk_tiles("Tx", NCE)
            Ty_c = chunk_tiles("Ty", NCE)

            # ---- one MLP+JVP layer step (emitted in waves) -----------------
            class Chain:
                def __init__(self, ci, pre1_fn, n_tangents, Ws, Bs, seed_cols,
                             finals, w=FT):
                    self.ci = ci
                    self.pre1_fn = pre1_fn
                    self.n_tangents = n_tangents
                    self.Ws, self.Bs = Ws, Bs
                    self.seed_cols = seed_cols
                    self.finals = finals
                    self.w = w
                    self.l1_mm = False
                    self.y = None
                    self.ts = [None] * n_tangents

            def emit_chain_layer(ch, li):
                ci, w = ch.ci, ch.w
                pre1_fn, n_tangents = ch.pre1_fn, ch.n_tangents
                Ws, Bs, seed_cols, finals = ch.Ws, ch.Bs, ch.seed_cols, ch.finals
                y, ts = ch.y, ch.ts
                if True:
                    # forward pre-activation
                    if li == 0:
                        if ch.l1_mm:
                            pre_ps = psp.tile([P, w], F32, name=f"p1_{ci}",
                                              tag="ps")
                            pre1_fn(pre_ps)
                            pre_ap = pre_ps[:]
                        else:
                            pre_sb = work.tile([P, w], F32, name=f"p1_{ci}",
                                               tag="pre1")
                            pre1_fn(pre_sb)
                            pre_ap = pre_sb[:]
                    else:
                        pre = psp.tile([P, w], F32, name=f"pre{ci}_{li}",
                                       tag="ps")
                        nc.tensor.matmul(pre[:], Ws[li], y[:],
                                         start=True, stop=True)
                        pre_ap = pre[:]
                    # tangent pre-products (layers >= 1)
                    tps = []
                    if li > 0:
                        for k in range(n_tangents):
                            tp = psp.tile([P, w], F32,
                                          name=f"tp{ci}_{li}_{k}", tag="ps")
                            nc.tensor.matmul(tp[:], Ws[li], ts[k][:],
                                             start=True, stop=True)
                            tps.append(tp)
                    # y = tanh(pre + b); on the last layer with no final
                    # scale, write straight into the destination tile
                    is_last_fwd_direct = (li == N_LAYERS - 1
                                          and finals[0][1] is None)
                    if is_last_fwd_direct:
                        y_new = None
                        y_ap = finals[0][0]
                    else:
                        y_new = work.tile([P, w], F32, name=f"y{ci}_{li}",
                                          tag="y")
                        y_ap = y_new[:]
                    nc.scalar.activation(y_ap, pre_ap, ACTF.Tanh, bias=Bs[li])
                    # d = 1 - y^2   (square on ACT, affine on DVE)
                    sq = work.tile([P, w], F32, name=f"sq{ci}_{li}", tag="sq")
                    nc.scalar.activation(sq[:], y_ap, ACTF.Square)
                    d = work.tile([P, w], F32, name=f"d{ci}_{li}", tag="d")
                    nc.vector.tensor_scalar(d[:], sq[:], -1.0, 1.0,
                                            ALU.mult, ALU.add)
                    # tangent updates
                    is_last = li == N_LAYERS - 1
                    t_new = []
                    for k in range(n_tangents):
                        if is_last:
                            tk, _scale = finals[1 + k]
                        else:
                            tk_t = work.tile([P, w], F32,
                                             name=f"t{ci}_{li}_{k}", tag=f"t{k}")
                            tk = tk_t[:]
                        if li == 0:
                            # tangent seed: d * seed_col_k
                            nc.vector.tensor_scalar(tk, d[:], seed_cols[k],
                                                    None, ALU.mult)
                        else:
                            nc.vector.tensor_mul(tk, tps[k][:], d[:])
                            if is_last and finals[1 + k][1] is not None:
                                nc.vector.tensor_scalar(tk, tk,
                                                        finals[1 + k][1], None,
                                                        ALU.mult)
                        if not is_last:
                            t_new.append(tk_t)
                    ts = t_new
                    if is_last:
                        dst, scale = finals[0]
                        if scale is not None:
                            nc.vector.tensor_scalar(dst, y_ap, scale,
                                                    None, ALU.mult)
                    y = y_new
                ch.y, ch.ts = y, ts

            # trunk chunks: pre1 = tW1.T @ coords_t (K=2 matmul, PE is idle
            # in the head window and the input is only 8KB)
            def trunk_chain(ci):
                sl = slice(ci * FT, (ci + 1) * FT)

                def trunk_pre1(dst, sl=sl):
                    nc.tensor.matmul(dst[:], tW1_sb[:], co_sb[:, sl],
                                     start=True, stop=True)

                c = Chain(f"t{ci}", trunk_pre1, 2, tW_sb, tB_sb,
                          [tW1xc_sb, tW1yc_sb],
                          [(Tb_c[ci][:], wlin_sb), (Tx_c[ci][:], wlin_sb),
                           (Ty_c[ci][:], wlin_sb)])
                c.l1_mm = True
                return c

            # branch chunks: pre1 = w1 * DT + b1
            def branch_chain(ci):
                sl = slice(ci * FT, (ci + 1) * FT)

                def branch_pre1(dst, sl=sl):
                    nc.scalar.activation(dst[:], dtb_sb[:, sl], ACTF.Identity,
                                         bias=bB_sb[0], scale=bW1c_sb)

                return Chain(f"b{ci}", branch_pre1, 1, bW_sb, bB_sb, [bW1c_sb],
                             [(A_c[ci][:], None), (Ap_c[ci][:], None)])

            # ---- main loop emitters: 4 outputs, [128 x 512] psum tiles -----
            rhs_for = {"U": Tb_c, "DX": Tx_c, "DY": Ty_c, "DMU": Tb_c}
            lhs_for = {"U": A_c, "DX": A_c, "DY": A_c, "DMU": Ap_c}
            order = ["U", "DX", "DY", "DMU"]
            evict_state = [0]

            def emit_main_bt(bt):
                bsl = slice((bt % 4) * P, (bt % 4 + 1) * P)
                lhs_ci = bt // 4
                for o in order:
                    lhsT = lhs_for[o][lhs_ci]
                    rhs = rhs_for[o]
                    st = stage.tile([P, NE], F32, name=f"st_{o}_{bt}", tag="st")
                    for nt in range(NCE):
                        ps_t = psp.tile([P, FT], F32,
                                        name=f"mm_{o}_{bt}_{nt}", tag="ps")
                        nc.tensor.matmul(ps_t[:], lhsT[:, bsl], rhs[nt][:],
                                         start=True, stop=True)
                        dstv = st[:, nt * FT:(nt + 1) * FT]
                        if evict_state[0] % 2 == 0:
                            nc.scalar.copy(dstv, ps_t[:])
                        else:
                            nc.vector.tensor_copy(dstv, ps_t[:])
                        evict_state[0] += 1
                        nc.sync.dma_start(
                            outs[o].ap()[bt * P:(bt + 1) * P,
                                         nt * FT:(nt + 1) * FT], dstv)

            # layer-wave emission: all chunks advance one layer per wave so
            # the in-order PE stream always has ready matmuls from other
            # chunks while one chunk's elementwise chain completes.  The
            # final branch layers are interleaved with the main tiles they
            # unblock.
            t_chains = [trunk_chain(ci) for ci in range(NCE)]
            b_chains = [branch_chain(ci) for ci in range(NCB)]
            chains = t_chains + b_chains
            for li in range(N_LAYERS - 1):
                for ch in chains:
                    emit_chain_layer(ch, li)
            for ch in t_chains:
                emit_chain_layer(ch, N_LAYERS - 1)
            for ci in range(NCB):
                emit_chain_layer(b_chains[ci], N_LAYERS - 1)
                for bt in range(4 * ci, 4 * ci + 4):
                    emit_main_bt(bt)

    nc.compile()
    return nc


def _get_nc(main_dt):
    if main_dt not in _CACHE:
        _CACHE[main_dt] = _build(main_dt)
    return _CACHE[main_dt]


def kernel(DT, coords, branch_Ws, branch_bs, trunk_Ws, trunk_bs, w_lin,
           _main_dt="f32", _want_results=False):
    DT = np.asarray(DT, np.float32)
    coords = np.asarray(coords, np.float32)
    nc = _get_nc(_main_dt)

    bWs = [np.asarray(w, np.float32) for w in branch_Ws]
    tWs = [np.asarray(w, np.float32) for w in trunk_Ws]
    bbs = [np.asarray(b, np.float32).reshape(P, 1) for b in branch_bs]
    tbs = [np.asarray(b, np.float32).reshape(P, 1) for b in trunk_bs]
    shared = {
        "tW1": np.ascontiguousarray(tWs[0]),
        "wpack": np.ascontiguousarray(
            np.concatenate(bWs[1:] + tWs[1:], axis=1)),
        "vpack": np.ascontiguousarray(np.concatenate(
            bbs + tbs + [bWs[0].T, np.asarray(w_lin, np.float32).reshape(P, 1),
                         tWs[0].T], axis=1)),
    }

    in_maps = []
    for c in range(RB * RE):
        rb, re = c // RE, c % RE
        m = dict(shared)
        m["dtb"] = np.ascontiguousarray(np.broadcast_to(
            DT[rb * NB:(rb + 1) * NB, :].T, (P, NB)))
        m["co_t"] = np.ascontiguousarray(coords[re * NE:(re + 1) * NE, :].T)
        in_maps.append(m)

    res = run_bass_kernel_spmd(nc, in_maps, core_ids=list(range(RB * RE)))

    full = {k: np.empty((B_FULL, E_FULL), np.float32)
            for k in ("U", "DX", "DY", "DMU")}
    for c in range(RB * RE):
        rb, re = c // RE, c % RE
        for k in full:
            full[k][rb * NB:(rb + 1) * NB, re * NE:(re + 1) * NE] = \
                res.results[c][k]
    out = tuple(full[k].reshape(B_FULL, E_FULL, 1)
                for k in ("U", "DX", "DY", "DMU"))
    if _want_results:
        return out, res
    return out
